# revision 29
# baseline (speedup 1.0000x reference)
"""Trainium2 Bass kernel for nn_Attention_84679575208344 (Performer-style
linear attention). Data-parallel over batch: 8 batches -> 8 NeuronCores.

Math per batch b (reference):
  qkv = x @ Wqkv.T -> split q,k,v per head (HD=48)
  qp = relu(dn*q)+1e-3 ; kp = relu(dn*k)+1e-3          (dn = 48**-0.25)
  ks = kp.sum(n) ; D = qp @ ks ; kptv = v.T @ kp (per head)
  attn = (qp @ kptv.T) / (D + 1e-8)
  out  = reshape(B,H,N,HD)->(B,N,C) WITHOUT head transpose, then @ Wproj.T + b

Design notes (v2, bf16):
 - x is converted to bf16 on the host; x^T comes straight from DRAM via the
   XBAR DMA-transpose (no PE transposes, no PSUM->SBUF copies for x^T).
 - v is never materialized: kptv_h = Wv_h @ (x^T @ kp_h).  G = x^T @ kp is
   accumulated on PE across all token chunks (G's lhsT is token-major x).
 - ks = column sums of kp via free-size-1 matmuls against a ones vector,
   accumulated into spare PSUM columns of the G tiles.
 - D is computed per 512-token block from the padded ks layout; 1/D is
   broadcast to 128 partitions with one fat SBUF->SBUF DMA per cc group and
   multiplied into qp in place (division-free attention afterwards).
 - Attention matmuls use 64-wide lhsT slices (48 real + 16 zero columns) so
   both j-parities of a (head, jj) pair land in one PSUM tile, fully
   initialized -> a single [128,512] copy builds the at-layout tile.
 - Projection consumes the 64-padded at layout (zero wp rows kill padding);
   bias is added on the host after the gather.
"""

from contextlib import ExitStack

import numpy as np

import concourse.bass as bass
import concourse.mybir as mybir
import concourse.tile as tile
from concourse import bacc

F32 = mybir.dt.float32
BF16 = mybir.dt.bfloat16
AL = mybir.AluOpType

B, N, C, H = 8, 4096, 384, 8
HD = 48
KEPS = 1e-3
DN = float(HD ** (-0.25))
NCHUNK = N // 128   # 32
NBLK = N // 512     # 8
NSLAB = N // 1024   # 4

_NC_CACHE = {}


def _rep_mid(src_ap, n):
    """Insert a zero-step middle dim: [p, F] -> [p, n(rep), F]."""
    return bass.AP(tensor=src_ap.tensor, offset=src_ap.offset,
                   ap=[src_ap.ap[0], [0, n], src_ap.ap[1]])


def build_nc():
    nc = bacc.Bacc("TRN2", target_bir_lowering=False, debug=False, num_devices=8)
    x = nc.declare_dram_parameter("x", [N, C], BF16, isOutput=False)
    wq = nc.declare_dram_parameter("wq", [C, 512], BF16, isOutput=False)
    wk = nc.declare_dram_parameter("wk", [C, C], BF16, isOutput=False)
    wvT = nc.declare_dram_parameter("wvT", [C, C], BF16, isOutput=False)
    wp = nc.declare_dram_parameter("wp", [512, C], BF16, isOutput=False)
    sel = nc.declare_dram_parameter("sel", [8, 4 * 128], BF16, isOutput=False)
    out = nc.declare_dram_parameter("out", [N, C], F32, isOutput=True)

    # engine rotations: tensor_scalar runs on DVE/GpSimd only; copies can
    # additionally use the Activation engine (scalar.copy)
    _ts_rot, _cp_rot = [0], [0]
    ts_engs = cp_engs = None

    def rot():
        e = ts_engs[_ts_rot[0] % len(ts_engs)]
        _ts_rot[0] += 1
        return e

    def rotc():
        e = cp_engs[_cp_rot[0] % len(cp_engs)]
        _cp_rot[0] += 1
        return e

    def copy_any(out, in_):
        e = rotc()
        if e is nc.scalar:
            e.copy(out=out, in_=in_)
        else:
            e.tensor_copy(out=out, in_=in_)

    with tile.TileContext(nc) as tc, ExitStack() as ctx:
        # NOTE: GPSIMD (Pool) cannot access PSUM on TRN2 -- walrus rejects it.
        # Everything reading PSUM must run on DVE or the Activation engine;
        # Pool only gets SBUF->SBUF work (the 1/D normalize muls).
        ts_engs = (nc.vector,)
        cp_engs = (nc.vector, nc.scalar)
        persist = ctx.enter_context(tc.tile_pool(name="persist", bufs=1))
        at_p = ctx.enter_context(tc.tile_pool(name="at", bufs=3))
        zo_p = ctx.enter_context(tc.tile_pool(name="zo", bufs=3))
        rbig_p = ctx.enter_context(tc.tile_pool(name="rbig", bufs=2))
        rbs_p = ctx.enter_context(tc.tile_pool(name="rbs", bufs=3))
        # PSUM: pa(2) + pb(2) + pg(3) + pk(1) = 8 banks
        pa = ctx.enter_context(tc.tile_pool(name="pa", bufs=2, space="PSUM"))
        pb = ctx.enter_context(tc.tile_pool(name="pb", bufs=2, space="PSUM"))
        pg = ctx.enter_context(tc.tile_pool(name="pg", bufs=1, space="PSUM"))
        pk = ctx.enter_context(tc.tile_pool(name="pk", bufs=1, space="PSUM"))

        xT = persist.tile([128, 3, N], BF16)
        xin = persist.tile([128, NCHUNK, C], BF16)
        kp_all = persist.tile([128, NCHUNK, C], BF16)
        qpT = persist.tile([128, 4, N], BF16)
        wq_sb = persist.tile([128, 3, 512], BF16)
        wk_sb = persist.tile([128, 3, C], BF16)
        wvT_sb = persist.tile([128, 3, C], BF16)
        wp_sb = persist.tile([128, 4, C], BF16)
        G_sb = persist.tile([128, 3, C], BF16)
        kptv_sb = persist.tile([128, 4, 128], BF16)
        ks_sb = persist.tile([128, 4, 8], BF16)
        ks_dense = persist.tile([128, 3], BF16)
        rd_sb = persist.tile([8, N], BF16)
        ones = persist.tile([128, 1], BF16)

        nc.vector.memset(ones[:], 1.0)
        nc.vector.memset(kptv_sb[:], 0.0)
        nc.vector.memset(ks_sb[:], 0.0)
        # warm-up: keep PE busy through its ~3us p-state ramp while the first
        # input DMAs are in flight (results are never read)
        for w in range(8):
            warm = pa.tile([1, 512], F32, tag="pa", name=f"warm{w}")
            nc.tensor.matmul(warm[:], ones[0:1, :], _rep_mid(ones[0:1, :], 512),
                             start=True, stop=True)

        # ---- input DMAs, ordered for earliest first compute:
        # wk + slab0 x^T unlock the k-GEMM; xin unlocks G; wq the q-GEMM;
        # wvT/wp are only needed in the tail.
        # Few, fat DMAs in strict priority order on the sync engine: each DMA
        # costs ~1-2us of issue latency (SEQ+HWDGE+DGE), so count matters more
        # than granularity.  wk + slab0 x^T unlock the k-GEMM first.
        def emit_xt(t0, t1):
            ts = slice(t0, t1)
            for cb in range(3):
                nc.sync.dma_start(out=xT[:, cb, ts],
                                  in_=x[ts, 128 * cb:128 * (cb + 1)],
                                  transpose=True)

        def emit_xin(t0, t1):
            nc.sync.dma_start(
                out=xin[:, t0 // 128:t1 // 128, :],
                in_=x[t0:t1, :].rearrange("(i p) c -> p i c", p=128))

        nc.sync.dma_start(out=wk_sb[:, 0, :], in_=wk[0:128, :])
        for cb in range(3):
            nc.sync.dma_start(out=xT[:, cb, 0:256],
                              in_=x[0:256, 128 * cb:128 * (cb + 1)],
                              transpose=True)
        nc.sync.dma_start(out=wk_sb[:, 1:3, :],
                          in_=wk[128:C, :].rearrange("(c p) d -> p c d", p=128))
        for cb in range(3):
            nc.sync.dma_start(out=xT[:, cb, 256:1024],
                              in_=x[256:1024, 128 * cb:128 * (cb + 1)],
                              transpose=True)
        emit_xin(0, 512)
        nc.sync.dma_start(out=wq_sb[:], in_=wq[:].rearrange("(c p) d -> p c d", p=128))
        emit_xin(512, 1024)
        emit_xt(1024, 2048)
        emit_xin(1024, 2048)
        emit_xt(2048, N)
        emit_xin(2048, N)
        nc.sync.dma_start(out=wvT_sb[:], in_=wvT[:].rearrange("(c p) d -> p c d", p=128))
        nc.sync.dma_start(out=wp_sb[:], in_=wp[:].rearrange("(a p) d -> p a d", p=128))

        # ---- phase B: k-GEMM, kp, G & ks accumulation, q-GEMM ----
        G_ps = [pg.tile([128, C], F32, tag=f"g{ct}", name=f"gps{ct}")
                for ct in range(3)]

        def emit_q_block(blk):
            bs = slice(512 * blk, 512 * (blk + 1))
            for mc in range(4):
                pq = pa.tile([128, 512], F32, tag="pa")
                for kc in range(3):
                    nc.tensor.matmul(pq[:], wq_sb[:, kc, 128 * mc:128 * (mc + 1)],
                                     xT[:, kc, bs],
                                     start=(kc == 0), stop=(kc == 2))
                rot().tensor_scalar(qpT[:, mc, bs], pq[:], 0.0, KEPS,
                                    op0=AL.max, op1=AL.add)

        def emit_g_chunk(i):
            for ct in range(3):
                nc.tensor.matmul(G_ps[ct][:], xin[:, i, 128 * ct:128 * (ct + 1)],
                                 kp_all[:, i, :],
                                 start=(i == 0), stop=(i == NCHUNK - 1))

        for i in range(NCHUNK):
            cs = slice(128 * i, 128 * (i + 1))
            # alternate PSUM banks (pb is idle during phase B) so the next
            # chunk's k-GEMM doesn't wait on this chunk's relu
            kpsum = (pk if i % 2 == 0 else pb).tile(
                [128, C], F32, tag="pk" if i % 2 == 0 else "pb", name=f"kps{i}")
            for kc in range(3):
                nc.tensor.matmul(kpsum[:], xT[:, kc, cs], wk_sb[:, kc, :],
                                 start=(kc == 0), stop=(kc == 2))
            rot().tensor_scalar(kp_all[:, i, :], kpsum[:], 0.0, KEPS,
                                op0=AL.max, op1=AL.add)
            if i > 0:
                emit_g_chunk(i - 1)     # G lags one chunk: hides the relu
            if i % 4 == 3 and i < 24:
                emit_q_block(i // 4)
        emit_g_chunk(NCHUNK - 1)

        # ---- tail: ks sums directly in the 64-padded layout ----
        # (48-wide lhsT slices of kp with 32-aligned output placement; one
        # sequential accumulation group per head to respect PSUM zero regions)
        kssum = pk.tile([128, 4], F32, tag="pk")
        for h in range(H):
            o = 64 * (h % 2)
            for i in range(NCHUNK):
                nc.tensor.matmul(kssum[o:o + 48, h // 2:h // 2 + 1],
                                 kp_all[:, i, 48 * h:48 * h + 48], ones[:],
                                 start=(i == 0), stop=(i == NCHUNK - 1),
                                 tile_position=(0, o))
        for h in range(H):
            o = 64 * (h % 2)
            nc.vector.tensor_copy(out=ks_sb[o:o + 48, h // 2, h:h + 1],
                                  in_=kssum[o:o + 48, h // 2:h // 2 + 1])
        for ct in range(3):
            copy_any(G_sb[:, ct, :], G_ps[ct][:])

        # kptv_h[m, d] = sum_c G[c, m] * WvT[c, d], duplicated into both
        # 64-col groups of kptv_sb (so j-even/j-odd lhsT slices exist)
        def emit_kptv(cc):
            kptvp = pg.tile([128, 128], F32, tag=f"g{cc % 3}", name=f"kptvp{cc}")
            for p in range(2):
                h = 2 * cc + p
                o = 64 * p
                for ct in range(3):
                    nc.tensor.matmul(kptvp[o:o + 48, o:o + 48],
                                     G_sb[:, ct, 48 * h:48 * h + 48],
                                     wvT_sb[:, ct, 48 * h:48 * h + 48],
                                     start=(ct == 0), stop=(ct == 2),
                                     tile_position=(0, o))
            for p in range(2):
                o = 64 * p
                dst = bass.AP(tensor=kptv_sb.tensor,
                              offset=kptv_sb[o:o + 48, cc, 0:48].offset,
                              ap=[kptv_sb[o:o + 48, cc, 0:48].ap[0], [64, 2],
                                  kptv_sb[o:o + 48, cc, 0:48].ap[1]])
                copy_any(dst, _rep_mid(kptvp[o:o + 48, o:o + 48], 2))

        # D per 512-token block; rd = 1/D  (D >= ~30, the 1e-8 eps is noise).
        # q-blocks 6/7 and the kptv matmuls are interleaved here as PE filler
        # while the 1/D chain (recip -> broadcast DMA -> normalize) drains.
        # sel_all[:, cc, :] is an [8,128] selector: out row p of the
        # broadcast matmul picks rd row 2cc (p<64) or 2cc+1 (p>=64)
        sel_all = persist.tile([8, 4, 128], BF16)
        nc.sync.dma_start(out=sel_all[:],
                           in_=sel[:].rearrange("p (a d) -> p a d", d=128))

        def emit_d_block(blk):
            bs = slice(512 * blk, 512 * (blk + 1))
            rdps = pb.tile([8, 512], F32, tag="pb")
            for cc in range(4):
                nc.tensor.matmul(rdps[:], ks_sb[:, cc, :], qpT[:, cc, bs],
                                 start=(cc == 0), stop=(cc == 3))
            with nc.allow_low_precision(reason="1/D to bf16 is plenty for 2e-2 tol"):
                nc.vector.reciprocal(rd_sb[:, bs], rdps[:])
            # normalize cc0/cc1 of this block range as soon as 1/D exists so
            # attention on heads 0-3 can start right after the last block.
            # Early blocks: pair-granular rep-DMA broadcast (cheap, latency
            # hidden); final blocks 6/7: PE row-broadcast (short chain).
            if blk == 5:
                ps = slice(0, 512 * 6)
                for cc in range(2):
                    rbs = rbs_p.tile([128, 512 * 6], BF16, tag="rbs",
                                     name=f"rbs{blk}_{cc}")
                    nc.sync.dma_start(
                        out=rbs[:], in_=_rep_mid(rd_sb[2 * cc:2 * cc + 2, ps], 64))
                    nc.gpsimd.tensor_tensor(out=qpT[:, cc, ps], in0=qpT[:, cc, ps],
                                            in1=rbs[:], op=AL.mult)
            elif blk >= 6:
                # last two blocks gate attention: shortest chain, all on DVE
                for cc in range(2):
                    rbp = pg.tile([128, 512], F32, tag=f"g{(2 * blk + cc) % 3}",
                                  name=f"rbp{blk}_{cc}")
                    nc.tensor.matmul(rbp[:], sel_all[:, cc, :], rd_sb[:, bs],
                                     start=True, stop=True)
                    nc.vector.tensor_tensor(out=qpT[:, cc, bs], in0=qpT[:, cc, bs],
                                            in1=rbp[:], op=AL.mult)

        emit_q_block(6)
        for blk in range(4):
            emit_d_block(blk)
        emit_kptv(0)
        emit_kptv(1)
        emit_q_block(7)
        for blk in range(4, NBLK):
            emit_d_block(blk)
        emit_kptv(2)
        emit_kptv(3)

        # broadcast 1/D rows across partitions and fold into qp (in place).
        # Whole-cc granularity on purpose: it depends on ALL recips, so the
        # scheduler cannot hoist it over the critical per-block reciprocals.
        for cc in range(2, 4):
            rbig = rbig_p.tile([128, N], BF16)
            nc.sync.dma_start(out=rbig[:],
                              in_=_rep_mid(rd_sb[2 * cc:2 * cc + 2, :], 64))
            nc.gpsimd.tensor_tensor(out=qpT[:, cc, :], in0=qpT[:, cc, :],
                                      in1=rbig[:], op=AL.mult)

        # ---- attention (at layout) + projection ----
        def emit_attn_head(h):
            cc, p = h // 2, h % 2
            o = 64 * p
            at = at_p.tile([128, 4, 512], BF16, tag="at")
            # qh[:, j, :] addresses tokens {8r+j} (stride-8 moving operand)
            qh = qpT[o:o + 48, cc, :].rearrange("p (r j) -> p j r", j=8)
            for jj in range(4):
                po = pa.tile([128, 512], F32, tag="pa")
                nc.tensor.matmul(po[0:64, :], kptv_sb[o:o + 48, cc, 0:64],
                                 qh[:, 2 * jj, :], start=True, stop=True,
                                 tile_position=(o, 0))
                nc.tensor.matmul(po[64:128, :], kptv_sb[o:o + 48, cc, 64:128],
                                 qh[:, 2 * jj + 1, :], start=True, stop=True,
                                 tile_position=(o, 64))
                if jj % 2 == 0:
                    nc.scalar.copy(out=at[:, jj, :], in_=po[:])
                else:
                    nc.vector.tensor_copy(out=at[:, jj, :], in_=po[:])
            return at

        zo_engs = (nc.scalar, nc.vector, nc.scalar, nc.vector)

        def emit_proj_head(h, at, split_store=False):
            zo = zo_p.tile([128, 4, C], F32)
            for rc in range(4):
                pz = pb.tile([128, C], F32, tag="pb")
                for jj in range(4):
                    nc.tensor.matmul(pz[:], at[:, jj, 128 * rc:128 * (rc + 1)],
                                     wp_sb[:, jj, :],
                                     start=(jj == 0), stop=(jj == 3))
                e = zo_engs[rc]
                if e is nc.scalar:
                    e.copy(out=zo[:, rc, :], in_=pz[:])
                else:
                    e.tensor_copy(out=zo[:, rc, :], in_=pz[:])
                if split_store:
                    r0 = 512 * h + 128 * rc
                    deng = nc.sync if rc % 2 == 0 else nc.scalar
                    deng.dma_start(out=out[r0:r0 + 128, :],
                                   in_=zo[:, rc, :])
            if not split_store:
                r0 = 512 * h
                nc.sync.dma_start(
                    out=out[r0:r0 + 512, :].rearrange("(a p) c -> p a c", p=128),
                    in_=zo[:])

        ats = {}
        ats[0] = emit_attn_head(0)
        for h in range(1, H):
            ats[h] = emit_attn_head(h)
            emit_proj_head(h - 1, ats.pop(h - 1), split_store=(h - 1 >= 6))
        emit_proj_head(H - 1, ats.pop(H - 1), split_store=True)
    nc.finalize()
    return nc


def _prep_inputs(inputs):
    """Host-side prep: bf16 casts, folded dn, padded/transposed weights.

    Returns (per-batch device input maps, bias) — bias is added on host."""
    import ml_dtypes

    x = np.asarray(inputs["x"], dtype=np.float32)
    Wqkv = np.asarray(inputs["Wqkv"], dtype=np.float32)
    Wproj = np.asarray(inputs["Wproj"], dtype=np.float32)
    bproj = np.asarray(inputs["bproj"], dtype=np.float32)

    Wq = Wqkv[0:C, :]
    Wk = Wqkv[C:2 * C, :]
    Wv = Wqkv[2 * C:3 * C, :]
    wq = np.zeros((C, 512), np.float32)
    for h in range(H):
        wq[:, 64 * h:64 * h + 48] = (DN * Wq[48 * h:48 * (h + 1), :]).T
    wk = (DN * Wk).T.copy()
    wvT = Wv.T.copy()
    wp = np.zeros((512, C), np.float32)
    WprojT = Wproj.T
    for j in range(8):
        wp[64 * j:64 * j + 48, :] = WprojT[48 * j:48 * (j + 1), :]

    sel = np.zeros((8, 4, 128), np.float32)
    for cc in range(4):
        sel[2 * cc, cc, 0:64] = 1.0
        sel[2 * cc + 1, cc, 64:128] = 1.0
    sel = sel.reshape(8, 4 * 128)

    bf = ml_dtypes.bfloat16
    consts = {"wq": wq.astype(bf), "wk": wk.astype(bf),
              "wvT": wvT.astype(bf), "wp": wp.astype(bf), "sel": sel.astype(bf)}
    in_maps = [dict(consts, x=np.ascontiguousarray(x[b]).astype(bf))
               for b in range(B)]
    return in_maps, bproj


def _run(inputs, trace=False):
    from concourse.bass_utils import run_bass_kernel_spmd

    in_maps, bproj = _prep_inputs(inputs)
    if "nc" not in _NC_CACHE:
        _NC_CACHE["nc"] = build_nc()
    nc = _NC_CACHE["nc"]
    res = run_bass_kernel_spmd(nc, in_maps, list(range(8)), trace=trace)
    out = np.stack([res.results[b]["out"] for b in range(B)], axis=0)
    out = out + bproj[None, None, :]
    return out.astype(np.float32), res


def kernel(**inputs) -> np.ndarray:
    out, _ = _run(inputs, trace=False)
    return out


def kernel_profiled(**inputs):
    out, res = _run(inputs, trace=True)
    return out, res


def sim_one_core(inputs, core=0):
    """CoreSim cost-model run of one core; returns (sim, output+bias)."""
    from concourse.bass_interp import CoreSim

    in_maps, bproj = _prep_inputs(inputs)
    nc = build_nc()
    sim = CoreSim(nc, publish_trace=False)
    for k, v in in_maps[core].items():
        sim.tensor(k)[:] = v
    sim.simulate()
    out = np.asarray(sim.tensor("out")) + bproj[None, :]
    return sim, out


# revision 30
# speedup vs baseline: 1.0282x; 1.0282x over previous
"""Trainium2 Bass kernel for nn_Attention_84679575208344 (Performer-style
linear attention). Data-parallel over batch: 8 batches -> 8 NeuronCores.

Math per batch b (reference):
  qkv = x @ Wqkv.T -> split q,k,v per head (HD=48)
  qp = relu(dn*q)+1e-3 ; kp = relu(dn*k)+1e-3          (dn = 48**-0.25)
  ks = kp.sum(n) ; D = qp @ ks ; kptv = v.T @ kp (per head)
  attn = (qp @ kptv.T) / (D + 1e-8)
  out  = reshape(B,H,N,HD)->(B,N,C) WITHOUT head transpose, then @ Wproj.T + b

Design notes (v2, bf16):
 - x is converted to bf16 on the host; x^T comes straight from DRAM via the
   XBAR DMA-transpose (no PE transposes, no PSUM->SBUF copies for x^T).
 - v is never materialized: kptv_h = Wv_h @ (x^T @ kp_h).  G = x^T @ kp is
   accumulated on PE across all token chunks (G's lhsT is token-major x).
 - ks = column sums of kp via free-size-1 matmuls against a ones vector,
   accumulated into spare PSUM columns of the G tiles.
 - D is computed per 512-token block from the padded ks layout; 1/D is
   broadcast to 128 partitions with one fat SBUF->SBUF DMA per cc group and
   multiplied into qp in place (division-free attention afterwards).
 - Attention matmuls use 64-wide lhsT slices (48 real + 16 zero columns) so
   both j-parities of a (head, jj) pair land in one PSUM tile, fully
   initialized -> a single [128,512] copy builds the at-layout tile.
 - Projection consumes the 64-padded at layout (zero wp rows kill padding);
   bias is added on the host after the gather.
"""

from contextlib import ExitStack

import numpy as np

import concourse.bass as bass
import concourse.mybir as mybir
import concourse.tile as tile
from concourse import bacc

F32 = mybir.dt.float32
BF16 = mybir.dt.bfloat16
AL = mybir.AluOpType

B, N, C, H = 8, 4096, 384, 8
HD = 48
KEPS = 1e-3
DN = float(HD ** (-0.25))
NCHUNK = N // 128   # 32
NBLK = N // 512     # 8
NSLAB = N // 1024   # 4

_NC_CACHE = {}


def _rep_mid(src_ap, n):
    """Insert a zero-step middle dim: [p, F] -> [p, n(rep), F]."""
    return bass.AP(tensor=src_ap.tensor, offset=src_ap.offset,
                   ap=[src_ap.ap[0], [0, n], src_ap.ap[1]])


def build_nc():
    nc = bacc.Bacc("TRN2", target_bir_lowering=False, debug=False, num_devices=8)
    x = nc.declare_dram_parameter("x", [N, C], BF16, isOutput=False)
    wq = nc.declare_dram_parameter("wq", [C, 512], BF16, isOutput=False)
    wk = nc.declare_dram_parameter("wk", [C, C], BF16, isOutput=False)
    wvT = nc.declare_dram_parameter("wvT", [C, C], BF16, isOutput=False)
    wp = nc.declare_dram_parameter("wp", [512, C], BF16, isOutput=False)
    sel = nc.declare_dram_parameter("sel", [8, 4 * 128], BF16, isOutput=False)
    out = nc.declare_dram_parameter("out", [N, C], F32, isOutput=True)

    # engine rotations: tensor_scalar runs on DVE/GpSimd only; copies can
    # additionally use the Activation engine (scalar.copy)
    _ts_rot, _cp_rot = [0], [0]
    ts_engs = cp_engs = None

    def rot():
        e = ts_engs[_ts_rot[0] % len(ts_engs)]
        _ts_rot[0] += 1
        return e

    def rotc():
        e = cp_engs[_cp_rot[0] % len(cp_engs)]
        _cp_rot[0] += 1
        return e

    def copy_any(out, in_):
        e = rotc()
        if e is nc.scalar:
            e.copy(out=out, in_=in_)
        else:
            e.tensor_copy(out=out, in_=in_)

    with tile.TileContext(nc) as tc, ExitStack() as ctx:
        # NOTE: GPSIMD (Pool) cannot access PSUM on TRN2 -- walrus rejects it.
        # Everything reading PSUM must run on DVE or the Activation engine;
        # Pool only gets SBUF->SBUF work (the 1/D normalize muls).
        ts_engs = (nc.vector,)
        cp_engs = (nc.vector, nc.scalar)
        persist = ctx.enter_context(tc.tile_pool(name="persist", bufs=1))
        at_p = ctx.enter_context(tc.tile_pool(name="at", bufs=3))
        zo_p = ctx.enter_context(tc.tile_pool(name="zo", bufs=3))
        rbig_p = ctx.enter_context(tc.tile_pool(name="rbig", bufs=2))
        rbs_p = ctx.enter_context(tc.tile_pool(name="rbs", bufs=3))
        # PSUM: pa(2) + pb(2) + pg(3) + pk(1) = 8 banks
        pa = ctx.enter_context(tc.tile_pool(name="pa", bufs=2, space="PSUM"))
        pb = ctx.enter_context(tc.tile_pool(name="pb", bufs=2, space="PSUM"))
        pg = ctx.enter_context(tc.tile_pool(name="pg", bufs=1, space="PSUM"))
        pk = ctx.enter_context(tc.tile_pool(name="pk", bufs=1, space="PSUM"))

        xT = persist.tile([128, 3, N], BF16)
        xin = persist.tile([128, NCHUNK, C], BF16)
        kp_all = persist.tile([128, NCHUNK, C], BF16)
        qpT = persist.tile([128, 4, N], BF16)
        wq_sb = persist.tile([128, 3, 512], BF16)
        wk_sb = persist.tile([128, 3, C], BF16)
        wvT_sb = persist.tile([128, 3, C], BF16)
        wp_sb = persist.tile([128, 4, C], BF16)
        G_sb = persist.tile([128, 3, C], BF16)
        kptv_sb = persist.tile([128, 4, 128], BF16)
        ks_sb = persist.tile([128, 4, 8], BF16)
        ks_dense = persist.tile([128, 3], BF16)
        rd_sb = persist.tile([8, N], BF16)
        ones = persist.tile([128, 1], BF16)

        nc.vector.memset(ones[:], 1.0)
        nc.vector.memset(kptv_sb[:], 0.0)
        nc.vector.memset(ks_sb[:], 0.0)
        # warm-up: keep PE busy through its ~3us p-state ramp while the first
        # input DMAs are in flight (results are never read)
        for w in range(8):
            warm = pa.tile([1, 512], F32, tag="pa", name=f"warm{w}")
            nc.tensor.matmul(warm[:], ones[0:1, :], _rep_mid(ones[0:1, :], 512),
                             start=True, stop=True)

        # ---- input DMAs, ordered for earliest first compute:
        # wk + slab0 x^T unlock the k-GEMM; xin unlocks G; wq the q-GEMM;
        # wvT/wp are only needed in the tail.
        # Few, fat DMAs in strict priority order on the sync engine: each DMA
        # costs ~1-2us of issue latency (SEQ+HWDGE+DGE), so count matters more
        # than granularity.  wk + slab0 x^T unlock the k-GEMM first.
        def emit_xt(t0, t1):
            ts = slice(t0, t1)
            for cb in range(3):
                nc.sync.dma_start(out=xT[:, cb, ts],
                                  in_=x[ts, 128 * cb:128 * (cb + 1)],
                                  transpose=True)

        def emit_xin(t0, t1):
            nc.sync.dma_start(
                out=xin[:, t0 // 128:t1 // 128, :],
                in_=x[t0:t1, :].rearrange("(i p) c -> p i c", p=128))

        nc.sync.dma_start(out=wk_sb[:, 0, :], in_=wk[0:128, :])
        for cb in range(3):
            nc.sync.dma_start(out=xT[:, cb, 0:256],
                              in_=x[0:256, 128 * cb:128 * (cb + 1)],
                              transpose=True)
        nc.sync.dma_start(out=wk_sb[:, 1:3, :],
                          in_=wk[128:C, :].rearrange("(c p) d -> p c d", p=128))
        for cb in range(3):
            nc.sync.dma_start(out=xT[:, cb, 256:1024],
                              in_=x[256:1024, 128 * cb:128 * (cb + 1)],
                              transpose=True)
        emit_xin(0, 512)
        nc.sync.dma_start(out=wq_sb[:], in_=wq[:].rearrange("(c p) d -> p c d", p=128))
        emit_xin(512, 1024)
        emit_xt(1024, 2048)
        emit_xin(1024, 2048)
        emit_xt(2048, N)
        emit_xin(2048, N)
        nc.sync.dma_start(out=wvT_sb[:], in_=wvT[:].rearrange("(c p) d -> p c d", p=128))
        nc.sync.dma_start(out=wp_sb[:], in_=wp[:].rearrange("(a p) d -> p a d", p=128))

        # ---- phase B: k-GEMM, kp, G & ks accumulation, q-GEMM ----
        G_ps = [pg.tile([128, C], F32, tag=f"g{ct}", name=f"gps{ct}")
                for ct in range(3)]

        def emit_q_block(blk):
            bs = slice(512 * blk, 512 * (blk + 1))
            for mc in range(4):
                pq = pa.tile([128, 512], F32, tag="pa")
                for kc in range(3):
                    nc.tensor.matmul(pq[:], wq_sb[:, kc, 128 * mc:128 * (mc + 1)],
                                     xT[:, kc, bs],
                                     start=(kc == 0), stop=(kc == 2))
                rot().tensor_scalar(qpT[:, mc, bs], pq[:], 0.0, KEPS,
                                    op0=AL.max, op1=AL.add)

        def emit_g_chunk(i):
            for ct in range(3):
                nc.tensor.matmul(G_ps[ct][:], xin[:, i, 128 * ct:128 * (ct + 1)],
                                 kp_all[:, i, :],
                                 start=(i == 0), stop=(i == NCHUNK - 1))

        for i in range(NCHUNK):
            cs = slice(128 * i, 128 * (i + 1))
            # alternate PSUM banks (pb is idle during phase B) so the next
            # chunk's k-GEMM doesn't wait on this chunk's relu
            kpsum = (pk if i % 2 == 0 else pb).tile(
                [128, C], F32, tag="pk" if i % 2 == 0 else "pb", name=f"kps{i}")
            for kc in range(3):
                nc.tensor.matmul(kpsum[:], xT[:, kc, cs], wk_sb[:, kc, :],
                                 start=(kc == 0), stop=(kc == 2))
            rot().tensor_scalar(kp_all[:, i, :], kpsum[:], 0.0, KEPS,
                                op0=AL.max, op1=AL.add)
            if i > 0:
                emit_g_chunk(i - 1)     # G lags one chunk: hides the relu
            if i % 4 == 3 and i < 24:
                emit_q_block(i // 4)
        emit_g_chunk(NCHUNK - 1)

        # ---- tail: ks sums directly in the 64-padded layout ----
        # (48-wide lhsT slices of kp with 32-aligned output placement; one
        # sequential accumulation group per head to respect PSUM zero regions)
        kssum = pk.tile([128, 4], F32, tag="pk")
        for h in range(H):
            o = 64 * (h % 2)
            for i in range(NCHUNK):
                nc.tensor.matmul(kssum[o:o + 48, h // 2:h // 2 + 1],
                                 kp_all[:, i, 48 * h:48 * h + 48], ones[:],
                                 start=(i == 0), stop=(i == NCHUNK - 1),
                                 tile_position=(0, o))
        for h in range(H):
            o = 64 * (h % 2)
            nc.vector.tensor_copy(out=ks_sb[o:o + 48, h // 2, h:h + 1],
                                  in_=kssum[o:o + 48, h // 2:h // 2 + 1])
        for ct in range(3):
            copy_any(G_sb[:, ct, :], G_ps[ct][:])

        # kptv_h[m, d] = sum_c G[c, m] * WvT[c, d], duplicated into both
        # 64-col groups of kptv_sb (so j-even/j-odd lhsT slices exist)
        def emit_kptv(cc):
            kptvp = pg.tile([128, 128], F32, tag=f"g{cc % 3}", name=f"kptvp{cc}")
            for p in range(2):
                h = 2 * cc + p
                o = 64 * p
                for ct in range(3):
                    nc.tensor.matmul(kptvp[o:o + 48, o:o + 48],
                                     G_sb[:, ct, 48 * h:48 * h + 48],
                                     wvT_sb[:, ct, 48 * h:48 * h + 48],
                                     start=(ct == 0), stop=(ct == 2),
                                     tile_position=(0, o))
            for p in range(2):
                o = 64 * p
                dst = bass.AP(tensor=kptv_sb.tensor,
                              offset=kptv_sb[o:o + 48, cc, 0:48].offset,
                              ap=[kptv_sb[o:o + 48, cc, 0:48].ap[0], [64, 2],
                                  kptv_sb[o:o + 48, cc, 0:48].ap[1]])
                copy_any(dst, _rep_mid(kptvp[o:o + 48, o:o + 48], 2))

        # D per 512-token block; rd = 1/D  (D >= ~30, the 1e-8 eps is noise).
        # q-blocks 6/7 and the kptv matmuls are interleaved here as PE filler
        # while the 1/D chain (recip -> broadcast DMA -> normalize) drains.
        # sel_all[:, cc, :] is an [8,128] selector: out row p of the
        # broadcast matmul picks rd row 2cc (p<64) or 2cc+1 (p>=64)
        sel_all = persist.tile([8, 4, 128], BF16)
        nc.sync.dma_start(out=sel_all[:],
                           in_=sel[:].rearrange("p (a d) -> p a d", d=128))

        def emit_d_block(blk):
            bs = slice(512 * blk, 512 * (blk + 1))
            rdps = pb.tile([8, 512], F32, tag="pb")
            for cc in range(4):
                nc.tensor.matmul(rdps[:], ks_sb[:, cc, :], qpT[:, cc, bs],
                                 start=(cc == 0), stop=(cc == 3))
            with nc.allow_low_precision(reason="1/D to bf16 is plenty for 2e-2 tol"):
                nc.vector.reciprocal(rd_sb[:, bs], rdps[:])
            # normalize cc0/cc1 of this block range as soon as 1/D exists so
            # attention on heads 0-3 can start right after the last block.
            # Early blocks: pair-granular rep-DMA broadcast (cheap, latency
            # hidden); final blocks 6/7: PE row-broadcast (short chain).
            if blk in (1, 3, 5):
                ps = slice(512 * (blk - 1), 512 * (blk + 1))
                for cc in range(2):
                    rbs = rbs_p.tile([128, 1024], BF16, tag="rbs",
                                     name=f"rbs{blk}_{cc}")
                    nc.sync.dma_start(
                        out=rbs[:], in_=_rep_mid(rd_sb[2 * cc:2 * cc + 2, ps], 64))
                    nc.gpsimd.tensor_tensor(out=qpT[:, cc, ps], in0=qpT[:, cc, ps],
                                            in1=rbs[:], op=AL.mult)
            elif blk >= 6:
                # last two blocks gate attention: shortest chain, all on DVE
                for cc in range(2):
                    rbp = pg.tile([128, 512], F32, tag=f"g{(2 * blk + cc) % 3}",
                                  name=f"rbp{blk}_{cc}")
                    nc.tensor.matmul(rbp[:], sel_all[:, cc, :], rd_sb[:, bs],
                                     start=True, stop=True)
                    nc.vector.tensor_tensor(out=qpT[:, cc, bs], in0=qpT[:, cc, bs],
                                            in1=rbp[:], op=AL.mult)

        emit_q_block(6)
        for blk in range(4):
            emit_d_block(blk)
        emit_kptv(0)
        emit_kptv(1)
        emit_q_block(7)
        for blk in range(4, NBLK):
            emit_d_block(blk)
        emit_kptv(2)
        emit_kptv(3)

        # broadcast 1/D rows across partitions and fold into qp (in place).
        # Whole-cc granularity on purpose: it depends on ALL recips, so the
        # scheduler cannot hoist it over the critical per-block reciprocals.
        for cc in range(2, 4):
            rbig = rbig_p.tile([128, N], BF16)
            nc.sync.dma_start(out=rbig[:],
                              in_=_rep_mid(rd_sb[2 * cc:2 * cc + 2, :], 64))
            nc.gpsimd.tensor_tensor(out=qpT[:, cc, :], in0=qpT[:, cc, :],
                                      in1=rbig[:], op=AL.mult)

        # ---- attention (at layout) + projection ----
        def emit_attn_head(h):
            cc, p = h // 2, h % 2
            o = 64 * p
            at = at_p.tile([128, 4, 512], BF16, tag="at")
            # qh[:, j, :] addresses tokens {8r+j} (stride-8 moving operand)
            qh = qpT[o:o + 48, cc, :].rearrange("p (r j) -> p j r", j=8)
            for jj in range(4):
                po = pa.tile([128, 512], F32, tag="pa")
                nc.tensor.matmul(po[0:64, :], kptv_sb[o:o + 48, cc, 0:64],
                                 qh[:, 2 * jj, :], start=True, stop=True,
                                 tile_position=(o, 0))
                nc.tensor.matmul(po[64:128, :], kptv_sb[o:o + 48, cc, 64:128],
                                 qh[:, 2 * jj + 1, :], start=True, stop=True,
                                 tile_position=(o, 64))
                if jj % 2 == 0:
                    nc.scalar.copy(out=at[:, jj, :], in_=po[:])
                else:
                    nc.vector.tensor_copy(out=at[:, jj, :], in_=po[:])
            return at

        zo_engs = (nc.scalar, nc.vector, nc.scalar, nc.vector)

        def emit_proj_head(h, at, split_store=False):
            zo = zo_p.tile([128, 4, C], F32)
            for rc in range(4):
                pz = pb.tile([128, C], F32, tag="pb")
                for jj in range(4):
                    nc.tensor.matmul(pz[:], at[:, jj, 128 * rc:128 * (rc + 1)],
                                     wp_sb[:, jj, :],
                                     start=(jj == 0), stop=(jj == 3))
                e = zo_engs[rc]
                if e is nc.scalar:
                    e.copy(out=zo[:, rc, :], in_=pz[:])
                else:
                    e.tensor_copy(out=zo[:, rc, :], in_=pz[:])
                if split_store:
                    r0 = 512 * h + 128 * rc
                    deng = nc.sync if rc % 2 == 0 else nc.scalar
                    deng.dma_start(out=out[r0:r0 + 128, :],
                                   in_=zo[:, rc, :])
            if not split_store:
                r0 = 512 * h
                nc.sync.dma_start(
                    out=out[r0:r0 + 512, :].rearrange("(a p) c -> p a c", p=128),
                    in_=zo[:])

        ats = {}
        ats[0] = emit_attn_head(0)
        for h in range(1, H):
            ats[h] = emit_attn_head(h)
            emit_proj_head(h - 1, ats.pop(h - 1), split_store=(h - 1 >= 6))
        emit_proj_head(H - 1, ats.pop(H - 1), split_store=True)
    nc.finalize()
    return nc


def _prep_inputs(inputs):
    """Host-side prep: bf16 casts, folded dn, padded/transposed weights.

    Returns (per-batch device input maps, bias) — bias is added on host."""
    import ml_dtypes

    x = np.asarray(inputs["x"], dtype=np.float32)
    Wqkv = np.asarray(inputs["Wqkv"], dtype=np.float32)
    Wproj = np.asarray(inputs["Wproj"], dtype=np.float32)
    bproj = np.asarray(inputs["bproj"], dtype=np.float32)

    Wq = Wqkv[0:C, :]
    Wk = Wqkv[C:2 * C, :]
    Wv = Wqkv[2 * C:3 * C, :]
    wq = np.zeros((C, 512), np.float32)
    for h in range(H):
        wq[:, 64 * h:64 * h + 48] = (DN * Wq[48 * h:48 * (h + 1), :]).T
    wk = (DN * Wk).T.copy()
    wvT = Wv.T.copy()
    wp = np.zeros((512, C), np.float32)
    WprojT = Wproj.T
    for j in range(8):
        wp[64 * j:64 * j + 48, :] = WprojT[48 * j:48 * (j + 1), :]

    sel = np.zeros((8, 4, 128), np.float32)
    for cc in range(4):
        sel[2 * cc, cc, 0:64] = 1.0
        sel[2 * cc + 1, cc, 64:128] = 1.0
    sel = sel.reshape(8, 4 * 128)

    bf = ml_dtypes.bfloat16
    consts = {"wq": wq.astype(bf), "wk": wk.astype(bf),
              "wvT": wvT.astype(bf), "wp": wp.astype(bf), "sel": sel.astype(bf)}
    in_maps = [dict(consts, x=np.ascontiguousarray(x[b]).astype(bf))
               for b in range(B)]
    return in_maps, bproj


def _run(inputs, trace=False):
    from concourse.bass_utils import run_bass_kernel_spmd

    in_maps, bproj = _prep_inputs(inputs)
    if "nc" not in _NC_CACHE:
        _NC_CACHE["nc"] = build_nc()
    nc = _NC_CACHE["nc"]
    res = run_bass_kernel_spmd(nc, in_maps, list(range(8)), trace=trace)
    out = np.stack([res.results[b]["out"] for b in range(B)], axis=0)
    out = out + bproj[None, None, :]
    return out.astype(np.float32), res


def kernel(**inputs) -> np.ndarray:
    out, _ = _run(inputs, trace=False)
    return out


def kernel_profiled(**inputs):
    out, res = _run(inputs, trace=True)
    return out, res


def sim_one_core(inputs, core=0):
    """CoreSim cost-model run of one core; returns (sim, output+bias)."""
    from concourse.bass_interp import CoreSim

    in_maps, bproj = _prep_inputs(inputs)
    nc = build_nc()
    sim = CoreSim(nc, publish_trace=False)
    for k, v in in_maps[core].items():
        sim.tensor(k)[:] = v
    sim.simulate()
    out = np.asarray(sim.tensor("out")) + bproj[None, :]
    return sim, out


# revision 31
# speedup vs baseline: 1.0345x; 1.0062x over previous
"""Trainium2 Bass kernel for nn_Attention_84679575208344 (Performer-style
linear attention). Data-parallel over batch: 8 batches -> 8 NeuronCores.

Math per batch b (reference):
  qkv = x @ Wqkv.T -> split q,k,v per head (HD=48)
  qp = relu(dn*q)+1e-3 ; kp = relu(dn*k)+1e-3          (dn = 48**-0.25)
  ks = kp.sum(n) ; D = qp @ ks ; kptv = v.T @ kp (per head)
  attn = (qp @ kptv.T) / (D + 1e-8)
  out  = reshape(B,H,N,HD)->(B,N,C) WITHOUT head transpose, then @ Wproj.T + b

Design notes (v2, bf16):
 - x is converted to bf16 on the host; x^T comes straight from DRAM via the
   XBAR DMA-transpose (no PE transposes, no PSUM->SBUF copies for x^T).
 - v is never materialized: kptv_h = Wv_h @ (x^T @ kp_h).  G = x^T @ kp is
   accumulated on PE across all token chunks (G's lhsT is token-major x).
 - ks = column sums of kp via free-size-1 matmuls against a ones vector,
   accumulated into spare PSUM columns of the G tiles.
 - D is computed per 512-token block from the padded ks layout; 1/D is
   broadcast to 128 partitions with one fat SBUF->SBUF DMA per cc group and
   multiplied into qp in place (division-free attention afterwards).
 - Attention matmuls use 64-wide lhsT slices (48 real + 16 zero columns) so
   both j-parities of a (head, jj) pair land in one PSUM tile, fully
   initialized -> a single [128,512] copy builds the at-layout tile.
 - Projection consumes the 64-padded at layout (zero wp rows kill padding);
   bias is added on the host after the gather.
"""

from contextlib import ExitStack

import numpy as np

import concourse.bass as bass
import concourse.mybir as mybir
import concourse.tile as tile
from concourse import bacc

F32 = mybir.dt.float32
BF16 = mybir.dt.bfloat16
AL = mybir.AluOpType

B, N, C, H = 8, 4096, 384, 8
HD = 48
KEPS = 1e-3
DN = float(HD ** (-0.25))
NCHUNK = N // 128   # 32
NBLK = N // 512     # 8
NSLAB = N // 1024   # 4

_NC_CACHE = {}


def _rep_mid(src_ap, n):
    """Insert a zero-step middle dim: [p, F] -> [p, n(rep), F]."""
    return bass.AP(tensor=src_ap.tensor, offset=src_ap.offset,
                   ap=[src_ap.ap[0], [0, n], src_ap.ap[1]])


def build_nc():
    nc = bacc.Bacc("TRN2", target_bir_lowering=False, debug=False, num_devices=8)
    x = nc.declare_dram_parameter("x", [N, C], BF16, isOutput=False)
    wq = nc.declare_dram_parameter("wq", [C, 512], BF16, isOutput=False)
    wk = nc.declare_dram_parameter("wk", [C, C], BF16, isOutput=False)
    wvT = nc.declare_dram_parameter("wvT", [C, C], BF16, isOutput=False)
    wp = nc.declare_dram_parameter("wp", [512, C], BF16, isOutput=False)
    sel = nc.declare_dram_parameter("sel", [8, 4 * 128], BF16, isOutput=False)
    out = nc.declare_dram_parameter("out", [N, C], F32, isOutput=True)

    # engine rotations: tensor_scalar runs on DVE/GpSimd only; copies can
    # additionally use the Activation engine (scalar.copy)
    _ts_rot, _cp_rot = [0], [0]
    ts_engs = cp_engs = None

    def rot():
        e = ts_engs[_ts_rot[0] % len(ts_engs)]
        _ts_rot[0] += 1
        return e

    def rotc():
        e = cp_engs[_cp_rot[0] % len(cp_engs)]
        _cp_rot[0] += 1
        return e

    def copy_any(out, in_):
        e = rotc()
        if e is nc.scalar:
            e.copy(out=out, in_=in_)
        else:
            e.tensor_copy(out=out, in_=in_)

    with tile.TileContext(nc) as tc, ExitStack() as ctx:
        # NOTE: GPSIMD (Pool) cannot access PSUM on TRN2 -- walrus rejects it.
        # Everything reading PSUM must run on DVE or the Activation engine;
        # Pool only gets SBUF->SBUF work (the 1/D normalize muls).
        ts_engs = (nc.vector,)
        cp_engs = (nc.vector, nc.scalar)
        persist = ctx.enter_context(tc.tile_pool(name="persist", bufs=1))
        at_p = ctx.enter_context(tc.tile_pool(name="at", bufs=3))
        zo_p = ctx.enter_context(tc.tile_pool(name="zo", bufs=3))
        rbig_p = ctx.enter_context(tc.tile_pool(name="rbig", bufs=2))
        rbs_p = ctx.enter_context(tc.tile_pool(name="rbs", bufs=3))
        # PSUM: pa(2) + pb(2) + pg(3) + pk(1) = 8 banks
        pa = ctx.enter_context(tc.tile_pool(name="pa", bufs=2, space="PSUM"))
        pb = ctx.enter_context(tc.tile_pool(name="pb", bufs=2, space="PSUM"))
        pg = ctx.enter_context(tc.tile_pool(name="pg", bufs=1, space="PSUM"))
        pk = ctx.enter_context(tc.tile_pool(name="pk", bufs=1, space="PSUM"))

        xT = persist.tile([128, 3, N], BF16)
        xin = persist.tile([128, NCHUNK, C], BF16)
        kp_all = persist.tile([128, NCHUNK, C], BF16)
        qpT = persist.tile([128, 4, N], BF16)
        wq_sb = persist.tile([128, 3, 512], BF16)
        wk_sb = persist.tile([128, 3, C], BF16)
        wvT_sb = persist.tile([128, 3, C], BF16)
        wp_sb = persist.tile([128, 4, C], BF16)
        G_sb = persist.tile([128, 3, C], BF16)
        kptv_sb = persist.tile([128, 4, 128], BF16)
        ks_sb = persist.tile([128, 4, 8], BF16)
        ks_dense = persist.tile([128, 3], BF16)
        rd_sb = persist.tile([8, N], BF16)
        ones = persist.tile([128, 1], BF16)

        nc.vector.memset(ones[:], 1.0)
        nc.vector.memset(kptv_sb[:], 0.0)
        nc.vector.memset(ks_sb[:], 0.0)
        # warm-up: keep PE busy through its ~3us p-state ramp while the first
        # input DMAs are in flight (results are never read)
        for w in range(8):
            warm = pa.tile([1, 512], F32, tag="pa", name=f"warm{w}")
            nc.tensor.matmul(warm[:], ones[0:1, :], _rep_mid(ones[0:1, :], 512),
                             start=True, stop=True)

        # ---- input DMAs, ordered for earliest first compute:
        # wk + slab0 x^T unlock the k-GEMM; xin unlocks G; wq the q-GEMM;
        # wvT/wp are only needed in the tail.
        # Few, fat DMAs in strict priority order on the sync engine: each DMA
        # costs ~1-2us of issue latency (SEQ+HWDGE+DGE), so count matters more
        # than granularity.  wk + slab0 x^T unlock the k-GEMM first.
        def emit_xt(t0, t1):
            ts = slice(t0, t1)
            for cb in range(3):
                nc.sync.dma_start(out=xT[:, cb, ts],
                                  in_=x[ts, 128 * cb:128 * (cb + 1)],
                                  transpose=True)

        def emit_xin(t0, t1):
            nc.sync.dma_start(
                out=xin[:, t0 // 128:t1 // 128, :],
                in_=x[t0:t1, :].rearrange("(i p) c -> p i c", p=128))

        nc.sync.dma_start(out=wk_sb[:, 0, :], in_=wk[0:128, :])
        for cb in range(3):
            nc.sync.dma_start(out=xT[:, cb, 0:256],
                              in_=x[0:256, 128 * cb:128 * (cb + 1)],
                              transpose=True)
        nc.sync.dma_start(out=wk_sb[:, 1:3, :],
                          in_=wk[128:C, :].rearrange("(c p) d -> p c d", p=128))
        for cb in range(3):
            nc.sync.dma_start(out=xT[:, cb, 256:1024],
                              in_=x[256:1024, 128 * cb:128 * (cb + 1)],
                              transpose=True)
        emit_xin(0, 512)
        nc.sync.dma_start(out=wq_sb[:], in_=wq[:].rearrange("(c p) d -> p c d", p=128))
        emit_xin(512, 1024)
        emit_xt(1024, 2048)
        emit_xin(1024, 2048)
        emit_xt(2048, N)
        emit_xin(2048, N)
        nc.sync.dma_start(out=wvT_sb[:], in_=wvT[:].rearrange("(c p) d -> p c d", p=128))
        nc.sync.dma_start(out=wp_sb[:], in_=wp[:].rearrange("(a p) d -> p a d", p=128))

        # ---- phase B: k-GEMM, kp, G & ks accumulation, q-GEMM ----
        G_ps = [pg.tile([128, C], F32, tag=f"g{ct}", name=f"gps{ct}")
                for ct in range(3)]

        def emit_q_block(blk):
            bs = slice(512 * blk, 512 * (blk + 1))
            for mc in range(4):
                pq = pa.tile([128, 512], F32, tag="pa")
                for kc in range(3):
                    nc.tensor.matmul(pq[:], wq_sb[:, kc, 128 * mc:128 * (mc + 1)],
                                     xT[:, kc, bs],
                                     start=(kc == 0), stop=(kc == 2))
                rot().tensor_scalar(qpT[:, mc, bs], pq[:], 0.0, KEPS,
                                    op0=AL.max, op1=AL.add)

        def emit_g_chunk(i):
            for ct in range(3):
                nc.tensor.matmul(G_ps[ct][:], xin[:, i, 128 * ct:128 * (ct + 1)],
                                 kp_all[:, i, :],
                                 start=(i == 0), stop=(i == NCHUNK - 1))

        for i in range(NCHUNK):
            cs = slice(128 * i, 128 * (i + 1))
            # alternate PSUM banks (pb is idle during phase B) so the next
            # chunk's k-GEMM doesn't wait on this chunk's relu
            kpsum = (pk if i % 2 == 0 else pb).tile(
                [128, C], F32, tag="pk" if i % 2 == 0 else "pb", name=f"kps{i}")
            for kc in range(3):
                nc.tensor.matmul(kpsum[:], xT[:, kc, cs], wk_sb[:, kc, :],
                                 start=(kc == 0), stop=(kc == 2))
            rot().tensor_scalar(kp_all[:, i, :], kpsum[:], 0.0, KEPS,
                                op0=AL.max, op1=AL.add)
            if i > 0:
                emit_g_chunk(i - 1)     # G lags one chunk: hides the relu
            if i % 4 == 3 and i < 24:
                emit_q_block(i // 4)
        emit_g_chunk(NCHUNK - 1)

        # ---- tail: ks sums directly in the 64-padded layout ----
        # (48-wide lhsT slices of kp with 32-aligned output placement; one
        # sequential accumulation group per head to respect PSUM zero regions)
        kssum = pk.tile([128, 4], F32, tag="pk")
        for h in range(H):
            o = 64 * (h % 2)
            for i in range(NCHUNK):
                nc.tensor.matmul(kssum[o:o + 48, h // 2:h // 2 + 1],
                                 kp_all[:, i, 48 * h:48 * h + 48], ones[:],
                                 start=(i == 0), stop=(i == NCHUNK - 1),
                                 tile_position=(0, o))
        for h in range(H):
            o = 64 * (h % 2)
            nc.vector.tensor_copy(out=ks_sb[o:o + 48, h // 2, h:h + 1],
                                  in_=kssum[o:o + 48, h // 2:h // 2 + 1])
        for ct in range(3):
            copy_any(G_sb[:, ct, :], G_ps[ct][:])

        # kptv_h[m, d] = sum_c G[c, m] * WvT[c, d], duplicated into both
        # 64-col groups of kptv_sb (so j-even/j-odd lhsT slices exist)
        def emit_kptv(cc):
            kptvp = pg.tile([128, 128], F32, tag=f"g{cc % 3}", name=f"kptvp{cc}")
            for p in range(2):
                h = 2 * cc + p
                o = 64 * p
                for ct in range(3):
                    nc.tensor.matmul(kptvp[o:o + 48, o:o + 48],
                                     G_sb[:, ct, 48 * h:48 * h + 48],
                                     wvT_sb[:, ct, 48 * h:48 * h + 48],
                                     start=(ct == 0), stop=(ct == 2),
                                     tile_position=(0, o))
            for p in range(2):
                o = 64 * p
                dst = bass.AP(tensor=kptv_sb.tensor,
                              offset=kptv_sb[o:o + 48, cc, 0:48].offset,
                              ap=[kptv_sb[o:o + 48, cc, 0:48].ap[0], [64, 2],
                                  kptv_sb[o:o + 48, cc, 0:48].ap[1]])
                copy_any(dst, _rep_mid(kptvp[o:o + 48, o:o + 48], 2))

        # D per 512-token block; rd = 1/D  (D >= ~30, the 1e-8 eps is noise).
        # q-blocks 6/7 and the kptv matmuls are interleaved here as PE filler
        # while the 1/D chain (recip -> broadcast DMA -> normalize) drains.
        # sel_all[:, cc, :] is an [8,128] selector: out row p of the
        # broadcast matmul picks rd row 2cc (p<64) or 2cc+1 (p>=64)
        sel_all = persist.tile([8, 4, 128], BF16)
        nc.sync.dma_start(out=sel_all[:],
                           in_=sel[:].rearrange("p (a d) -> p a d", d=128))

        def emit_d_block(blk):
            bs = slice(512 * blk, 512 * (blk + 1))
            rdps = pb.tile([8, 512], F32, tag="pb")
            for cc in range(4):
                nc.tensor.matmul(rdps[:], ks_sb[:, cc, :], qpT[:, cc, bs],
                                 start=(cc == 0), stop=(cc == 3))
            with nc.allow_low_precision(reason="1/D to bf16 is plenty for 2e-2 tol"):
                nc.vector.reciprocal(rd_sb[:, bs], rdps[:])
            # normalize cc0/cc1 of this block range as soon as 1/D exists so
            # attention on heads 0-3 can start right after the last block.
            # Early blocks: pair-granular rep-DMA broadcast (cheap, latency
            # hidden); final blocks 6/7: PE row-broadcast (short chain).
            if blk in (1, 3, 5):
                ps = slice(512 * (blk - 1), 512 * (blk + 1))
                for cc in range(2):
                    rbs = rbs_p.tile([128, 1024], BF16, tag="rbs",
                                     name=f"rbs{blk}_{cc}")
                    nc.sync.dma_start(
                        out=rbs[:], in_=_rep_mid(rd_sb[2 * cc:2 * cc + 2, ps], 64))
                    nc.gpsimd.tensor_tensor(out=qpT[:, cc, ps], in0=qpT[:, cc, ps],
                                            in1=rbs[:], op=AL.mult)
            elif blk >= 6:
                # last two blocks gate attention: shortest chain, all on DVE,
                # each broadcast in its own PSUM bank (no recycling waits)
                for cc in range(2):
                    k = 2 * (blk - 6) + cc
                    rbp = (pg if k < 3 else pk).tile(
                        [128, 512], F32, tag=f"g{k}" if k < 3 else "pk",
                        name=f"rbp{blk}_{cc}")
                    nc.tensor.matmul(rbp[:], sel_all[:, cc, :], rd_sb[:, bs],
                                     start=True, stop=True)
                    nc.vector.tensor_tensor(out=qpT[:, cc, bs], in0=qpT[:, cc, bs],
                                            in1=rbp[:], op=AL.mult)

        emit_q_block(6)
        for blk in range(4):
            emit_d_block(blk)
        emit_kptv(0)
        emit_kptv(1)
        emit_q_block(7)
        for blk in range(4, NBLK):
            emit_d_block(blk)
        emit_kptv(2)
        emit_kptv(3)

        # broadcast 1/D rows across partitions and fold into qp (in place).
        # Whole-cc granularity on purpose: it depends on ALL recips, so the
        # scheduler cannot hoist it over the critical per-block reciprocals.
        for cc in range(2, 4):
            rbig = rbig_p.tile([128, N], BF16)
            nc.sync.dma_start(out=rbig[:],
                              in_=_rep_mid(rd_sb[2 * cc:2 * cc + 2, :], 64))
            nc.gpsimd.tensor_tensor(out=qpT[:, cc, :], in0=qpT[:, cc, :],
                                      in1=rbig[:], op=AL.mult)

        # ---- attention (at layout) + projection ----
        def emit_attn_head(h):
            cc, p = h // 2, h % 2
            o = 64 * p
            at = at_p.tile([128, 4, 512], BF16, tag="at")
            # qh[:, j, :] addresses tokens {8r+j} (stride-8 moving operand)
            qh = qpT[o:o + 48, cc, :].rearrange("p (r j) -> p j r", j=8)
            for jj in range(4):
                po = pa.tile([128, 512], F32, tag="pa")
                nc.tensor.matmul(po[0:64, :], kptv_sb[o:o + 48, cc, 0:64],
                                 qh[:, 2 * jj, :], start=True, stop=True,
                                 tile_position=(o, 0))
                nc.tensor.matmul(po[64:128, :], kptv_sb[o:o + 48, cc, 64:128],
                                 qh[:, 2 * jj + 1, :], start=True, stop=True,
                                 tile_position=(o, 64))
                if jj % 2 == 0:
                    nc.scalar.copy(out=at[:, jj, :], in_=po[:])
                else:
                    nc.vector.tensor_copy(out=at[:, jj, :], in_=po[:])
            return at

        zo_engs = (nc.scalar, nc.vector, nc.scalar, nc.vector)

        def emit_proj_head(h, at, split_store=False):
            zo = zo_p.tile([128, 4, C], F32)
            for rc in range(4):
                pz = (pb if rc % 2 == 0 else pg).tile(
                    [128, C], F32, tag="pb" if rc % 2 == 0 else f"g{rc % 3}",
                    name=f"pz{h}_{rc}")
                for jj in range(4):
                    nc.tensor.matmul(pz[:], at[:, jj, 128 * rc:128 * (rc + 1)],
                                     wp_sb[:, jj, :],
                                     start=(jj == 0), stop=(jj == 3))
                e = zo_engs[rc]
                if e is nc.scalar:
                    e.copy(out=zo[:, rc, :], in_=pz[:])
                else:
                    e.tensor_copy(out=zo[:, rc, :], in_=pz[:])
                if split_store:
                    r0 = 512 * h + 128 * rc
                    deng = nc.sync if rc % 2 == 0 else nc.scalar
                    deng.dma_start(out=out[r0:r0 + 128, :],
                                   in_=zo[:, rc, :])
            if not split_store:
                r0 = 512 * h
                nc.sync.dma_start(
                    out=out[r0:r0 + 512, :].rearrange("(a p) c -> p a c", p=128),
                    in_=zo[:])

        ats = {}
        ats[0] = emit_attn_head(0)
        for h in range(1, H):
            ats[h] = emit_attn_head(h)
            emit_proj_head(h - 1, ats.pop(h - 1), split_store=(h - 1 >= 6))
        emit_proj_head(H - 1, ats.pop(H - 1), split_store=True)
    nc.finalize()
    return nc


def _prep_inputs(inputs):
    """Host-side prep: bf16 casts, folded dn, padded/transposed weights.

    Returns (per-batch device input maps, bias) — bias is added on host."""
    import ml_dtypes

    x = np.asarray(inputs["x"], dtype=np.float32)
    Wqkv = np.asarray(inputs["Wqkv"], dtype=np.float32)
    Wproj = np.asarray(inputs["Wproj"], dtype=np.float32)
    bproj = np.asarray(inputs["bproj"], dtype=np.float32)

    Wq = Wqkv[0:C, :]
    Wk = Wqkv[C:2 * C, :]
    Wv = Wqkv[2 * C:3 * C, :]
    wq = np.zeros((C, 512), np.float32)
    for h in range(H):
        wq[:, 64 * h:64 * h + 48] = (DN * Wq[48 * h:48 * (h + 1), :]).T
    wk = (DN * Wk).T.copy()
    wvT = Wv.T.copy()
    wp = np.zeros((512, C), np.float32)
    WprojT = Wproj.T
    for j in range(8):
        wp[64 * j:64 * j + 48, :] = WprojT[48 * j:48 * (j + 1), :]

    sel = np.zeros((8, 4, 128), np.float32)
    for cc in range(4):
        sel[2 * cc, cc, 0:64] = 1.0
        sel[2 * cc + 1, cc, 64:128] = 1.0
    sel = sel.reshape(8, 4 * 128)

    bf = ml_dtypes.bfloat16
    consts = {"wq": wq.astype(bf), "wk": wk.astype(bf),
              "wvT": wvT.astype(bf), "wp": wp.astype(bf), "sel": sel.astype(bf)}
    in_maps = [dict(consts, x=np.ascontiguousarray(x[b]).astype(bf))
               for b in range(B)]
    return in_maps, bproj


def _run(inputs, trace=False):
    from concourse.bass_utils import run_bass_kernel_spmd

    in_maps, bproj = _prep_inputs(inputs)
    if "nc" not in _NC_CACHE:
        _NC_CACHE["nc"] = build_nc()
    nc = _NC_CACHE["nc"]
    res = run_bass_kernel_spmd(nc, in_maps, list(range(8)), trace=trace)
    out = np.stack([res.results[b]["out"] for b in range(B)], axis=0)
    out = out + bproj[None, None, :]
    return out.astype(np.float32), res


def kernel(**inputs) -> np.ndarray:
    out, _ = _run(inputs, trace=False)
    return out


def kernel_profiled(**inputs):
    out, res = _run(inputs, trace=True)
    return out, res


def sim_one_core(inputs, core=0):
    """CoreSim cost-model run of one core; returns (sim, output+bias)."""
    from concourse.bass_interp import CoreSim

    in_maps, bproj = _prep_inputs(inputs)
    nc = build_nc()
    sim = CoreSim(nc, publish_trace=False)
    for k, v in in_maps[core].items():
        sim.tensor(k)[:] = v
    sim.simulate()
    out = np.asarray(sim.tensor("out")) + bproj[None, :]
    return sim, out
